# revision 1
# baseline (speedup 1.0000x reference)
"""Trainium2 Bass kernel for the VQ commitment-loss problem (fp8 DoubleRow).

Math
----
reference loss = 0.25 * mean((codebook[argmin_k dist] - flat)**2)
               = 0.25/(B*T*D) * sum_n min_k ||flat_n - e_k||^2
since the gathered quantized row realizes exactly the min squared distance.

min_k ||f - e||^2 = ||f||^2 + min_k (||e_k||^2 - 2 f.e_k)

Per core (2 of 16 batches):
  - sum_n ||f_n||^2 via the window-count trick:
        sum over tau of cnt(tau) * x_pad[tau]^2   (cnt = #windows containing tau)
  - the min term via fp8e4 DoubleRow TensorE matmuls (256-deep contraction
    per pass): window tiles [128, 4sub, T] are the stationary operand, the
    codebook scaled by -2 is the moving operand [128, 4sub, 1024].
    ||e_k||^2 rides as three extra contraction rows (32*r0 + r1 + r2 fp8
    decomposition, paired with a 32/1/1 column in the window operand).
    Two 128-window subtiles share a 4-bank PSUM tile; one VectorE 3D
    min-reduce [128,2,1024] -> [128,2] drains it.

All window data is expanded up-front into two resident [128, 4, 4096] SBUF
tiles through the SWDGE (gpsimd) queue — it spreads packets over all 16
SDMA engines, unlike the dynamic HWDGE rings which only engage ~3 for this
pattern — in staged waves so the main loop starts early.  Single-queue FIFO
keeps DMA completion monotone so shared completion-semaphore lanes cannot
alias a later DMA into an early matmul's wait.

Host side only pads/casts/shards inputs and sums the 8 per-core partials.
"""

import numpy as np
import ml_dtypes

B, P, T = 16, 12, 4096
WIN = 41
PAD = (WIN - 1) // 2          # 20
K = 1024
D = P * WIN                   # 492
COMMITMENT_COST = 0.25

NCORES = 8
BC = B // NCORES              # batches per core = 2
TP = T + 2 * PAD              # padded time = 4136
NCHUNK = 4                    # contraction subtiles: 3 pellets * 41 taps = 123 rows
CHROWS = 3 * WIN              # 123
NSUB = BC * T // 128          # 64 subtiles of 128 windows per core
NPAIR = NSUB // 2             # 32 PSUM pair-tiles
TCHUNK = TP // 4              # 1034 (xsq layout)
NWARM = 24                    # HAM warmup matmuls (bridge PE to main-loop start)

SCALE = COMMITMENT_COST / (B * T * D)

FP8NP = ml_dtypes.float8_e4m3

_CACHED = {}


def _build_nc():
    import concourse.bacc as bacc
    import concourse.bass as bass
    import concourse.mybir as mybir
    import concourse.tile as tile

    BF = mybir.dt.bfloat16
    F32 = mybir.dt.float32
    F8 = mybir.dt.float8e4
    AX = mybir.AxisListType
    OP = mybir.AluOpType
    DR = mybir.MatmulPerfMode.DoubleRow

    nc = bacc.Bacc("TRN2", target_bir_lowering=False, debug=False)

    xw_d = nc.dram_tensor("xw", [BC, P, TP], F8, kind="ExternalInput")
    cb_d = nc.dram_tensor("cb", [128, NCHUNK, K], F8, kind="ExternalInput")
    cnt_d = nc.dram_tensor("cnt", [96, TCHUNK], BF, kind="ExternalInput")
    ones_d = nc.dram_tensor("ones5", [5, T], F8, kind="ExternalInput")
    out_d = nc.dram_tensor("out", [1, 1], F32, kind="ExternalOutput")

    with tile.TileContext(nc) as tc:
        with (
            tc.tile_pool(name="cbpool", bufs=1) as cbpool,
            tc.tile_pool(name="wpool", bufs=1) as wpool,
            tc.tile_pool(name="misc", bufs=1) as misc,
        ):
            # ---- HAM warmup: PE busy from t~0 so the clock is 2.4 GHz when
            # the real matmuls start.
            warm_src = misc.tile([128, 512], BF)
            nc.vector.memset(warm_src[:], 0.5)
            with tc.tile_pool(name="pwarm", bufs=1, space="PSUM") as pwarm:
                wps = pwarm.tile([128, 512], F32)
                for _ in range(NWARM):
                    nc.tensor.matmul(
                        wps[:], warm_src[:, 0:128], warm_src[:], start=True, stop=True
                    )

            # ---- resident codebook tile [k, subtile, code]
            cbt = cbpool.tile([128, NCHUNK, K], F8)
            nc.gpsimd.dma_start(cbt[:], cb_d[:])

            ones_bf = misc.tile([128, 1], BF)
            nc.vector.memset(ones_bf[:], 1.0)
            ones_f = misc.tile([128, 1], F32)
            nc.vector.memset(ones_f[:], 1.0)
            mins_buf = misc.tile([128, NSUB], F32)

            # ---- resident window tiles wt[b]: [128, sub, T] fp8 with
            # wt[b][k, c, t] = xw[b, 3c + k//41, t + k%41] for k < 123.
            wt = [
                wpool.tile([128, NCHUNK, T], F8, tag=f"w{b}", name=f"wt{b}")
                for b in range(BC)
            ]

            def wslice_dma(c, b, lo, hi, eng=None):
                (eng or nc.gpsimd).dma_start(
                    wt[b][0:CHROWS, c, lo:hi],
                    bass.AP(
                        xw_d,
                        (b * P + 3 * c) * TP + lo,
                        [[TP, 3], [1, WIN], [1, hi - lo]],
                    ),
                )

            # ones rows for every window tile, on the scalar ring up front
            # (rows 123..127 get [32, 1, 1, 1, 1] from the host constant)
            for b in range(BC):
                for c in range(NCHUNK):
                    nc.scalar.dma_start(wt[b][CHROWS:128, c, :], ones_d[:])

            # batch-0 expansion waves (after cb on the same ring); the first
            # wave is sized so its last chunk lands with the codebook-norm
            # chain, and within each wave c2/c3 go first to match the
            # jp2-first matmul rotation
            for lo, hi in ((0, 1536), (1536, 2560), (2560, 3584), (3584, T)):
                for c in (2, 3, 0, 1):
                    wslice_dma(c, 0, lo, hi)

            # ---- prologue: c_k = ||e_k||^2 as fp8 rows 32*r0 + r1 + r2
            # into cbt rows 123..125 of subtile 0
            with (
                tc.tile_pool(name="pre", bufs=1) as pre,
                tc.tile_pool(name="ppre", bufs=1, space="PSUM") as ppre,
            ):
                sq = pre.tile([128, NCHUNK, K], BF)
                nc.vector.tensor_mul(sq[:], cbt[:], cbt[:])  # (-2e)^2 = 4 e^2
                # ones column scaled by 0.25 folds the (-2)^2 correction into
                # the matmul itself: pc = sum 0.25*sq = ||e||^2
                oquart = misc.tile([128, 1], BF)
                nc.vector.memset(oquart[:], 0.25)
                pc = ppre.tile([1, K], F32)
                for h in range(2):
                    for c in range(NCHUNK):
                        nc.tensor.matmul(
                            pc[:, 512 * h : 512 * (h + 1)],
                            oquart[:],
                            sq[:, c, 512 * h : 512 * (h + 1)],
                            start=(c == 0),
                            stop=(c == NCHUNK - 1),
                        )
                # spacer warmups: keep HAM hot across the prologue->main gap
                wps2 = ppre.tile([128, 512], F32)
                for _ in range(10):
                    nc.tensor.matmul(
                        wps2[:], warm_src[:, 0:128], warm_src[:], start=True, stop=True
                    )
                r0 = pre.tile([1, K], F8)
                nc.vector.tensor_scalar_mul(r0[:], pc[:], 1.0 / 32.0)
                r0f = pre.tile([1, K], F32)
                nc.vector.tensor_copy(r0f[:], r0[:])
                t1 = pre.tile([1, K], F32)
                nc.vector.tensor_scalar_mul(t1[:], r0f[:], 32.0)
                rem1 = pre.tile([1, K], F32)
                nc.vector.tensor_sub(rem1[:], pc[:], t1[:])
                r1 = pre.tile([1, K], F8)
                nc.vector.tensor_copy(r1[:], rem1[:])
                nc.sync.dma_start(cbt[CHROWS : CHROWS + 1, 0, :], r0[:])
                nc.sync.dma_start(cbt[CHROWS + 1 : CHROWS + 2, 0, :], r1[:])

            # batch-1 expansion waves (gpsimd ring, after batch 0)
            for lo, hi in ((0, 2048), (2048, T)):
                for c in range(NCHUNK):
                    wslice_dma(c, 1, lo, hi)

            # ---- xsq/cnt loads (scalar ring)
            xsq_in = misc.tile([96, TCHUNK], F8)
            nc.scalar.dma_start(
                xsq_in[:],
                bass.AP(
                    xw_d,
                    0,
                    [[P * TP, BC], [TP, P], [TCHUNK, 4], [1, TCHUNK]],
                ),
            )
            cnt_sb = misc.tile([96, TCHUNK], BF)
            nc.scalar.dma_start(cnt_sb[:], cnt_d[:])

            # ||f||^2 term, emitted BEFORE the main loop: the DVE is idle
            # between the norm chain and the first pair-reduce, whereas inside
            # the main loop it is saturated and these ops would push the last
            # reduce out by their full duration.
            sqx = misc.tile([96, TCHUNK], BF)
            wsq = misc.tile([96, TCHUNK], F32)
            selfsum = misc.tile([96, 1], F32)
            nc.vector.tensor_mul(sqx[:], xsq_in[:], xsq_in[:])
            nc.vector.tensor_mul(wsq[:], sqx[:], cnt_sb[:])
            nc.vector.tensor_reduce(selfsum[:], wsq[:], axis=AX.X, op=OP.add)

            # ---- main loop: 32 pairs of 128-window subtiles
            with tc.tile_pool(name="pmain", bufs=2, space="PSUM") as pmain:
                for pair in range(NPAIR):
                    ps = pmain.tile([128, 2, K], F32, tag="ps", name=f"ps_{pair}")
                    for s in range(2):
                        i = pair * 2 + s            # subtile index
                        b = i // (NSUB // BC)
                        toff = (i % (NSUB // BC)) * 128
                        for h in range(2):
                            # subtile pair (0,1) last: it carries the
                            # codebook-norm rows, which are ready latest
                            for jp in (2, 0):
                                nc.tensor.matmul(
                                    ps[:, s, 512 * h : 512 * (h + 1)],
                                    wt[b][:, jp : jp + 2, toff : toff + 128],
                                    cbt[:, jp : jp + 2, 512 * h : 512 * (h + 1)],
                                    start=(jp == 2),
                                    stop=(jp == 0),
                                    perf_mode=DR,
                                )
                    nc.vector.tensor_reduce(
                        mins_buf[:, 2 * pair : 2 * pair + 2],
                        ps[:],
                        axis=AX.X,
                        op=OP.min,
                    )

            # ---- finale: grand sum -> scale -> out
            macc = misc.tile([128, 1], F32)
            nc.vector.tensor_reduce(macc[:], mins_buf[:], axis=AX.X, op=OP.add)
            with tc.tile_pool(name="pfin", bufs=1, space="PSUM") as pfin:
                fin = pfin.tile([1, 1], F32)
                nc.tensor.matmul(fin[:], macc[:], ones_f[:], start=True, stop=False)
                nc.tensor.matmul(
                    fin[:], selfsum[:], ones_f[0:96, :], start=False, stop=True
                )
                res = misc.tile([1, 1], F32)
                nc.vector.tensor_scalar_mul(res[:], fin[:], float(SCALE))
                nc.gpsimd.dma_start(out_d[:], res[:])

    nc.compile()
    return nc


def get_nc():
    if "nc" not in _CACHED:
        _CACHED["nc"] = _build_nc()
    return _CACHED["nc"]


def _host_prep(x, codebook):
    """Pad/cast/shard the inputs; returns per-core in_maps."""
    x = np.asarray(x, dtype=np.float32)
    codebook = np.asarray(codebook, dtype=np.float32)

    x8 = x.astype(FP8NP)
    xw = np.zeros((B, P, TP), dtype=FP8NP)
    xw[:, :, PAD : PAD + T] = x8

    # value of the fp8-rounded codebook, exactly scaled by -2
    cbb = codebook.astype(FP8NP).astype(np.float32)
    rhs = np.zeros((128, NCHUNK, K), dtype=np.float32)
    for c in range(NCHUNK):
        rhs[:CHROWS, c, :] = -2.0 * cbb[:, CHROWS * c : CHROWS * (c + 1)].T
    rhs8 = rhs.astype(FP8NP)

    tau = np.arange(TP, dtype=np.float32)
    cnt = np.minimum(np.minimum(tau + 1.0, float(WIN)), float(TP) - tau)
    cnt_rep = np.tile(cnt.reshape(4, TCHUNK), (BC * P, 1)).astype(ml_dtypes.bfloat16)

    ones5 = np.ones((5, T), dtype=FP8NP)
    ones5[0, :] = FP8NP(32.0)

    in_maps = []
    for i in range(NCORES):
        in_maps.append(
            {
                "xw": np.ascontiguousarray(xw[BC * i : BC * (i + 1)]),
                "cb": rhs8,
                "cnt": cnt_rep,
                "ones5": ones5,
            }
        )
    return in_maps


def kernel(x, codebook):
    from concourse.bass_utils import run_bass_kernel_spmd

    nc = get_nc()
    in_maps = _host_prep(x, codebook)
    res = run_bass_kernel_spmd(nc, in_maps, core_ids=list(range(NCORES)))
    total = np.float64(0.0)
    for r in res.results:
        total += np.float64(r["out"][0, 0])
    return np.array(np.float32(total))



# revision 6
# speedup vs baseline: 1.1405x; 1.1405x over previous
"""Trainium2 Bass kernel for the VQ commitment-loss problem (fp8 DoubleRow).

Math
----
reference loss = 0.25 * mean((codebook[argmin_k dist] - flat)**2)
               = 0.25/(B*T*D) * sum_n min_k ||flat_n - e_k||^2
since the gathered quantized row realizes exactly the min squared distance.

min_k ||f - e||^2 = ||f||^2 + min_k (||e_k||^2 - 2 f.e_k)

The ||f||^2 term is a tiny O(B*P*T) reduction of the (fp8-rounded) input,
computed on the host via the window-count trick.  The device computes only
the dominant O(N*K*D) term: per core (2 of 16 batches)

  - sum_n min_k (||e_k||^2 - 2 f_n.e_k) via fp8e4 DoubleRow TensorE matmuls
    (256-deep contraction per pass): window tiles [128, 4sub, T] are the
    stationary operand, the codebook scaled by -2 is the moving operand
    [128, 4sub, 1024].  ||e_k||^2 rides as three extra contraction rows
    (32*r0 + r1 + r2 fp8 decomposition, precomputed on the host, paired
    with a [32,1,1,1,1] column in the window operand).
  - the [128, 2, 1024] PSUM pair-panels are drained by a split pipeline:
    most pairs are converted f32->fp16 into SBUF by the otherwise-idle
    ScalarE (activation Copy), then min-reduced by VectorE as a fp16
    tensor_tensor min tree running in the 2x_1p DVE mode; every 5th pair
    is reduced directly from PSUM in f32 by VectorE.  This balances the
    elementwise drain (~64 KB/pair) across both engines so neither exceeds
    the TensorE pace of ~1.8us/pair.

All window data is expanded up-front into two resident [128, 4, 4096] SBUF
tiles through the SWDGE (gpsimd) queue — it spreads packets over all 16
SDMA engines, unlike the dynamic HWDGE rings which only engage ~3 for this
pattern — in staged waves so the main loop starts early.  Single-queue FIFO
keeps DMA completion monotone so shared completion-semaphore lanes cannot
alias a later DMA into an early matmul's wait.  The scalar ring carries the
constant ones rows and the final store, keeping the gpsimd DGE drain off
the critical tail.

Host side pads/casts/shards inputs, precomputes the codebook norm rows and
the ||f||^2 self term, and sums the 8 per-core partials.
"""

import numpy as np
import ml_dtypes

B, P, T = 16, 12, 4096
WIN = 41
PAD = (WIN - 1) // 2          # 20
K = 1024
D = P * WIN                   # 492
COMMITMENT_COST = 0.25

NCORES = 8
BC = B // NCORES              # batches per core = 2
TP = T + 2 * PAD              # padded time = 4136
NCHUNK = 4                    # contraction subtiles: 3 pellets * 41 taps = 123 rows
CHROWS = 3 * WIN              # 123
NSUB = BC * T // 128          # 64 subtiles of 128 windows per core
NPAIR = NSUB // 2             # 32 PSUM pair-tiles
NWARM = 12                    # HAM warmup matmuls (bridge PE to main-loop start)

SCALE = COMMITMENT_COST / (B * T * D)

FP8NP = ml_dtypes.float8_e4m3

_CACHED = {}


def _build_nc():
    import concourse.bacc as bacc
    import concourse.bass as bass
    import concourse.mybir as mybir
    import concourse.tile as tile

    BF = mybir.dt.bfloat16
    F32 = mybir.dt.float32
    F16 = mybir.dt.float16
    F8 = mybir.dt.float8e4
    AX = mybir.AxisListType
    OP = mybir.AluOpType
    ACT = mybir.ActivationFunctionType
    DR = mybir.MatmulPerfMode.DoubleRow

    nc = bacc.Bacc("TRN2", target_bir_lowering=False, debug=False)

    xw_d = nc.dram_tensor("xw", [BC, P, TP], F8, kind="ExternalInput")
    cb_d = nc.dram_tensor("cb", [128, NCHUNK, K], F8, kind="ExternalInput")
    ones_d = nc.dram_tensor("ones5", [5, NCHUNK * T], F8, kind="ExternalInput")
    out_d = nc.dram_tensor("out", [1, 1], F32, kind="ExternalOutput")

    with tile.TileContext(nc) as tc:
        with (
            tc.tile_pool(name="cbpool", bufs=1) as cbpool,
            tc.tile_pool(name="wpool", bufs=1) as wpool,
            tc.tile_pool(name="misc", bufs=1) as misc,
        ):
            # ---- resident codebook tile [k, subtile, code]; rows 123..125 of
            # subtile 0 carry the host-precomputed ||e||^2 fp8 decomposition
            cbt = cbpool.tile([128, NCHUNK, K], F8)
            nc.gpsimd.dma_start(cbt[:], cb_d[:])

            # ---- resident window tiles wt[b]: [128, sub, T] fp8 with
            # wt[b][k, c, t] = xw[b, 3c + k//41, t + k%41] for k < 123.
            wt = [
                wpool.tile([128, NCHUNK, T], F8, tag=f"w{b}", name=f"wt{b}")
                for b in range(BC)
            ]

            def wslice_dma(c, b, lo, hi):
                nc.gpsimd.dma_start(
                    wt[b][0:CHROWS, c, lo:hi],
                    bass.AP(
                        xw_d,
                        (b * P + 3 * c) * TP + lo,
                        [[TP, 3], [1, WIN], [1, hi - lo]],
                    ),
                )

            # ones rows [32,1,1,1,1] for every window tile on the scalar ring
            for b in range(BC):
                nc.scalar.dma_start(wt[b][CHROWS:128, :, :], ones_d[:])

            # batch-0 expansion waves, then batch 1 in one slab; all behind
            # the codebook on the single SWDGE FIFO
            for lo, hi in ((0, 1024), (1024, 2560), (2560, T)):
                for c in range(NCHUNK):
                    wslice_dma(c, 0, lo, hi)
            for c in range(NCHUNK):
                wslice_dma(c, 1, 0, T)

            warm_src = misc.tile([128, 512], BF)
            nc.vector.memset(warm_src[:], 0.5)
            ones_f = misc.tile([128, 1], F32)
            nc.vector.memset(ones_f[:], 1.0)
            mins_buf = misc.tile([128, NSUB], F32)

            # ---- HAM warmup: PE busy early so the clock is 2.4 GHz when the
            # real matmuls start.
            with tc.tile_pool(name="pwarm", bufs=1, space="PSUM") as pwarm:
                wps = pwarm.tile([128, 512], F32)
                for _ in range(NWARM):
                    nc.tensor.matmul(
                        wps[:], warm_src[:, 0:128], warm_src[:], start=True, stop=True
                    )

            # ---- main loop: 32 pairs of 128-window subtiles
            with (
                tc.tile_pool(name="pmain", bufs=2, space="PSUM") as pmain,
                tc.tile_pool(name="drain", bufs=2) as drain,
            ):
                for pair in range(NPAIR):
                    ps = pmain.tile([128, 2, K], F32, tag="ps", name=f"ps_{pair}")
                    for s in range(2):
                        i = pair * 2 + s            # subtile index
                        b = i // (NSUB // BC)
                        toff = (i % (NSUB // BC)) * 128
                        for jp in (0, 2):
                            for h in range(2):
                                nc.tensor.matmul(
                                    ps[:, s, 512 * h : 512 * (h + 1)],
                                    wt[b][:, jp : jp + 2, toff : toff + 128],
                                    cbt[:, jp : jp + 2, 512 * h : 512 * (h + 1)],
                                    start=(jp == 0),
                                    stop=(jp == 2),
                                    perf_mode=DR,
                                )
                    mb = mins_buf[:, 2 * pair : 2 * pair + 2]
                    if pair % 5 == 2:
                        # direct f32 drain on VectorE
                        nc.vector.tensor_reduce(mb, ps[:], axis=AX.X, op=OP.min)
                    else:
                        # ScalarE converts to fp16; VectorE min-tree at 2x
                        sb16 = drain.tile([128, 2, K], F16, tag="sb16")
                        nc.scalar.activation(sb16[:], ps[:], ACT.Copy)
                        m512 = drain.tile([128, 2, 512], F16, tag="m512")
                        m256 = drain.tile([128, 2, 256], F16, tag="m256")
                        m128 = drain.tile([128, 2, 128], F16, tag="m128")
                        nc.vector.tensor_tensor(
                            m512[:], sb16[:, :, 0:512], sb16[:, :, 512:1024], op=OP.min
                        )
                        nc.vector.tensor_tensor(
                            m256[:], m512[:, :, 0:256], m512[:, :, 256:512], op=OP.min
                        )
                        nc.vector.tensor_tensor(
                            m128[:], m256[:, :, 0:128], m256[:, :, 128:256], op=OP.min
                        )
                        nc.vector.tensor_reduce(mb, m128[:], axis=AX.X, op=OP.min)

            # ---- finale: grand sum -> out (scaling + self term on host)
            macc = misc.tile([128, 1], F32)
            nc.vector.tensor_reduce(macc[:], mins_buf[:], axis=AX.X, op=OP.add)
            with tc.tile_pool(name="pfin", bufs=1, space="PSUM") as pfin:
                fin = pfin.tile([1, 1], F32)
                nc.tensor.matmul(fin[:], macc[:], ones_f[:], start=True, stop=True)
                res = misc.tile([1, 1], F32)
                nc.vector.tensor_copy(res[:], fin[:])
                nc.scalar.dma_start(out_d[:], res[:])

    nc.compile()
    return nc


def get_nc():
    if "nc" not in _CACHED:
        _CACHED["nc"] = _build_nc()
    return _CACHED["nc"]


def _host_prep(x, codebook):
    """Pad/cast/shard the inputs; returns (per-core in_maps, self term)."""
    x = np.asarray(x, dtype=np.float32)
    codebook = np.asarray(codebook, dtype=np.float32)

    x8 = x.astype(FP8NP)
    xw = np.zeros((B, P, TP), dtype=FP8NP)
    xw[:, :, PAD : PAD + T] = x8

    # value of the fp8-rounded codebook, exactly scaled by -2
    cbb = codebook.astype(FP8NP).astype(np.float32)
    rhs = np.zeros((128, NCHUNK, K), dtype=np.float32)
    for c in range(NCHUNK):
        rhs[:CHROWS, c, :] = -2.0 * cbb[:, CHROWS * c : CHROWS * (c + 1)].T
    rhs8 = rhs.astype(FP8NP)

    # ||e||^2 rows: c = 32*r0 + r1 + r2 in fp8, paired with the [32,1,1,1,1]
    # ones rows of the window tiles
    cnorm = (cbb.astype(np.float64) ** 2).sum(axis=1).astype(np.float32)
    r0 = (cnorm / 32.0).astype(FP8NP)
    rem1 = cnorm - 32.0 * r0.astype(np.float32)
    r1 = rem1.astype(FP8NP)
    rem2 = rem1 - r1.astype(np.float32)
    r2 = rem2.astype(FP8NP)
    rhs8[CHROWS, 0, :] = r0
    rhs8[CHROWS + 1, 0, :] = r1
    rhs8[CHROWS + 2, 0, :] = r2

    ones20 = np.ones((5, NCHUNK * T), dtype=FP8NP)
    ones20[0, :] = FP8NP(32.0)

    # host-side ||f||^2 term via the window-count trick
    tau = np.arange(TP, dtype=np.float64)
    cnt = np.minimum(np.minimum(tau + 1.0, float(WIN)), float(TP) - tau)
    xf = xw.astype(np.float64)
    self_term = float((xf * xf * cnt[None, None, :]).sum())

    in_maps = []
    for i in range(NCORES):
        in_maps.append(
            {
                "xw": np.ascontiguousarray(xw[BC * i : BC * (i + 1)]),
                "cb": rhs8,
                "ones5": ones20,
            }
        )
    return in_maps, self_term


def kernel(x, codebook):
    from concourse.bass_utils import run_bass_kernel_spmd

    nc = get_nc()
    in_maps, self_term = _host_prep(x, codebook)
    res = run_bass_kernel_spmd(nc, in_maps, core_ids=list(range(NCORES)))
    total = np.float64(self_term)
    for r in res.results:
        total += np.float64(r["out"][0, 0])
    return np.array(np.float32(SCALE * total))


# revision 10
# speedup vs baseline: 1.1764x; 1.0315x over previous
"""Trainium2 Bass kernel for the VQ commitment-loss problem (fp8 DoubleRow).

Math
----
reference loss = 0.25 * mean((codebook[argmin_k dist] - flat)**2)
               = 0.25/(B*T*D) * sum_n min_k ||flat_n - e_k||^2
since the gathered quantized row realizes exactly the min squared distance.

min_k ||f - e||^2 = ||f||^2 + min_k (||e_k||^2 - 2 f.e_k)

The ||f||^2 term is a tiny O(B*P*T) reduction of the (fp8-rounded) input,
computed on the host via the window-count trick.  The device computes only
the dominant O(N*K*D) term: per core (2 of 16 batches)

  - sum_n min_k (||e_k||^2 - 2 f_n.e_k) via fp8e4 DoubleRow TensorE matmuls
    (256-deep contraction per pass): window tiles [128, 4sub, T] are the
    stationary operand, the codebook scaled by -2 is the moving operand
    [128, 4sub, 1024].  ||e_k||^2 rides as three extra contraction rows
    (32*r0 + r1 + r2 fp8 decomposition, precomputed on the host, paired
    with a [32,1,1,1,1] column in the window operand).
  - the [128, 2, 1024] PSUM pair-panels are drained by a split pipeline:
    most pairs are converted f32->fp16 into SBUF by the otherwise-idle
    ScalarE (activation Copy), then min-reduced by VectorE as a fp16
    tensor_tensor min tree running in the 2x_1p DVE mode; every 5th pair
    is reduced directly from PSUM in f32 by VectorE.  This balances the
    elementwise drain (~64 KB/pair) across both engines so neither exceeds
    the TensorE pace of ~1.8us/pair.

All window data is expanded up-front into two resident [128, 4, 4096] SBUF
tiles through the SWDGE (gpsimd) queue — it spreads packets over all 16
SDMA engines, unlike the dynamic HWDGE rings which only engage ~3 for this
pattern — in staged waves so the main loop starts early.  Single-queue FIFO
keeps DMA completion monotone so shared completion-semaphore lanes cannot
alias a later DMA into an early matmul's wait.  The scalar ring carries the
constant ones rows and the final store, keeping the gpsimd DGE drain off
the critical tail.

Host side pads/casts/shards inputs, precomputes the codebook norm rows and
the ||f||^2 self term, and sums the 8 per-core partials.
"""

import numpy as np
import ml_dtypes

B, P, T = 16, 12, 4096
WIN = 41
PAD = (WIN - 1) // 2          # 20
K = 1024
D = P * WIN                   # 492
COMMITMENT_COST = 0.25

NCORES = 8
BC = B // NCORES              # batches per core = 2
TP = T + 2 * PAD              # padded time = 4136
NCHUNK = 4                    # contraction subtiles: 3 pellets * 41 taps = 123 rows
CHROWS = 3 * WIN              # 123
NSUB = BC * T // 128          # 64 subtiles of 128 windows per core
NPAIR = NSUB // 2             # 32 PSUM pair-tiles
NWARM = 13                    # HAM warmup matmuls (bridge PE to main-loop start)
DIRECT_PAIRS = (10, 21)       # pairs drained in f32 by VectorE (ACT catch-up)

SCALE = COMMITMENT_COST / (B * T * D)

FP8NP = ml_dtypes.float8_e4m3

_CACHED = {}


def _build_nc():
    import concourse.bacc as bacc
    import concourse.bass as bass
    import concourse.mybir as mybir
    import concourse.tile as tile

    BF = mybir.dt.bfloat16
    F32 = mybir.dt.float32
    F16 = mybir.dt.float16
    F8 = mybir.dt.float8e4
    AX = mybir.AxisListType
    OP = mybir.AluOpType
    ACT = mybir.ActivationFunctionType
    DR = mybir.MatmulPerfMode.DoubleRow

    nc = bacc.Bacc("TRN2", target_bir_lowering=False, debug=False)

    xw_d = nc.dram_tensor("xw", [BC, P, TP], F8, kind="ExternalInput")
    cb_d = nc.dram_tensor("cb", [128, NCHUNK, K], F8, kind="ExternalInput")
    ones_d = nc.dram_tensor("ones5", [5, NCHUNK * T], F8, kind="ExternalInput")
    out_d = nc.dram_tensor("out", [1, 1], F32, kind="ExternalOutput")

    with tile.TileContext(nc) as tc:
        with (
            tc.tile_pool(name="cbpool", bufs=1) as cbpool,
            tc.tile_pool(name="wpool", bufs=1) as wpool,
            tc.tile_pool(name="misc", bufs=1) as misc,
        ):
            # ---- resident codebook tile [k, subtile, code]; rows 123..125 of
            # subtile 0 carry the host-precomputed ||e||^2 fp8 decomposition
            cbt = cbpool.tile([128, NCHUNK, K], F8)

            # ---- resident window tiles wt[b]: [128, sub, T] fp8 with
            # wt[b][k, c, t] = xw[b, 3c + k//41, t + k%41] for k < 123.
            wt = [
                wpool.tile([128, NCHUNK, T], F8, tag=f"w{b}", name=f"wt{b}")
                for b in range(BC)
            ]

            def wslice_dma(c, b, lo, hi):
                nc.gpsimd.dma_start(
                    wt[b][0:CHROWS, c, lo:hi],
                    bass.AP(
                        xw_d,
                        (b * P + 3 * c) * TP + lo,
                        [[TP, 3], [1, WIN], [1, hi - lo]],
                    ),
                )

            # ones rows [32,1,1,1,1] for every window tile on the scalar ring
            for b in range(BC):
                nc.scalar.dma_start(wt[b][CHROWS:128, :, :], ones_d[:])

            # SWDGE FIFO order = landing order: the codebook chunk-pairs are
            # interleaved with a narrow first wave so pair 0's operands land
            # as early as possible; then wider waves, then batch 1.
            nc.gpsimd.dma_start(cbt[:, 0:2, :], cb_d[:, 0:2, :])
            for c in range(NCHUNK):
                wslice_dma(c, 0, 0, 512)
            nc.gpsimd.dma_start(cbt[:, 2:4, :], cb_d[:, 2:4, :])
            for lo, hi in ((512, 1536), (1536, T)):
                for c in range(NCHUNK):
                    wslice_dma(c, 0, lo, hi)
            for c in range(NCHUNK):
                wslice_dma(c, 1, 0, T)

            warm_src = misc.tile([128, 512], BF)
            nc.vector.memset(warm_src[:], 0.5)
            ones_f = misc.tile([128, 1], F32)
            nc.vector.memset(ones_f[:], 1.0)
            mins_buf = misc.tile([128, NSUB], F32)

            # ---- HAM warmup: PE busy early so the clock is 2.4 GHz when the
            # real matmuls start.
            with tc.tile_pool(name="pwarm", bufs=1, space="PSUM") as pwarm:
                wps = pwarm.tile([128, 512], F32)
                for _ in range(NWARM):
                    nc.tensor.matmul(
                        wps[:], warm_src[:, 0:128], warm_src[:], start=True, stop=True
                    )

            # ---- main loop: 32 pairs of 128-window subtiles
            with (
                tc.tile_pool(name="pmain", bufs=2, space="PSUM") as pmain,
                tc.tile_pool(name="drain", bufs=2) as drain,
            ):
                for pair in range(NPAIR):
                    ps = pmain.tile([128, 2, K], F32, tag="ps", name=f"ps_{pair}")
                    for s in range(2):
                        i = pair * 2 + s            # subtile index
                        b = i // (NSUB // BC)
                        toff = (i % (NSUB // BC)) * 128
                        for jp in (0, 2):
                            for h in range(2):
                                nc.tensor.matmul(
                                    ps[:, s, 512 * h : 512 * (h + 1)],
                                    wt[b][:, jp : jp + 2, toff : toff + 128],
                                    cbt[:, jp : jp + 2, 512 * h : 512 * (h + 1)],
                                    start=(jp == 0),
                                    stop=(jp == 2),
                                    perf_mode=DR,
                                )
                        if pair == NPAIR - 1:
                            # last pair: per-subtile f32 drain so the final
                            # reduce overlaps the other subtile's matmuls
                            nc.vector.tensor_reduce(
                                mins_buf[:, i : i + 1],
                                ps[:, s, :],
                                axis=AX.X,
                                op=OP.min,
                            )
                    if pair == NPAIR - 1:
                        pass
                    elif pair in DIRECT_PAIRS:
                        # direct f32 drain on VectorE (ACT catch-up slot)
                        nc.vector.tensor_reduce(
                            mins_buf[:, 2 * pair : 2 * pair + 2],
                            ps[:],
                            axis=AX.X,
                            op=OP.min,
                        )
                    else:
                        # ScalarE converts to fp16; VectorE min-tree at 2x
                        sb16 = drain.tile([128, 2, K], F16, tag="sb16")
                        nc.scalar.activation(sb16[:], ps[:], ACT.Copy)
                        m512 = drain.tile([128, 2, 512], F16, tag="m512")
                        m256 = drain.tile([128, 2, 256], F16, tag="m256")
                        m128 = drain.tile([128, 2, 128], F16, tag="m128")
                        nc.vector.tensor_tensor(
                            m512[:], sb16[:, :, 0:512], sb16[:, :, 512:1024], op=OP.min
                        )
                        nc.vector.tensor_tensor(
                            m256[:], m512[:, :, 0:256], m512[:, :, 256:512], op=OP.min
                        )
                        nc.vector.tensor_tensor(
                            m128[:], m256[:, :, 0:128], m256[:, :, 128:256], op=OP.min
                        )
                        nc.vector.tensor_reduce(
                            mins_buf[:, 2 * pair : 2 * pair + 2],
                            m128[:],
                            axis=AX.X,
                            op=OP.min,
                        )

            # ---- finale: grand sum -> out (scaling + self term on host)
            macc = misc.tile([128, 1], F32)
            nc.vector.tensor_reduce(macc[:], mins_buf[:], axis=AX.X, op=OP.add)
            with tc.tile_pool(name="pfin", bufs=1, space="PSUM") as pfin:
                fin = pfin.tile([1, 1], F32)
                nc.tensor.matmul(fin[:], macc[:], ones_f[:], start=True, stop=True)
                res = misc.tile([1, 1], F32)
                nc.vector.tensor_copy(res[:], fin[:])
                nc.scalar.dma_start(out_d[:], res[:])

    nc.compile()
    return nc


def get_nc():
    if "nc" not in _CACHED:
        _CACHED["nc"] = _build_nc()
    return _CACHED["nc"]


def _host_prep(x, codebook):
    """Pad/cast/shard the inputs; returns (per-core in_maps, self term)."""
    x = np.asarray(x, dtype=np.float32)
    codebook = np.asarray(codebook, dtype=np.float32)

    x8 = x.astype(FP8NP)
    xw = np.zeros((B, P, TP), dtype=FP8NP)
    xw[:, :, PAD : PAD + T] = x8

    # value of the fp8-rounded codebook, exactly scaled by -2
    cbb = codebook.astype(FP8NP).astype(np.float32)
    rhs = np.zeros((128, NCHUNK, K), dtype=np.float32)
    for c in range(NCHUNK):
        rhs[:CHROWS, c, :] = -2.0 * cbb[:, CHROWS * c : CHROWS * (c + 1)].T
    rhs8 = rhs.astype(FP8NP)

    # ||e||^2 rows: c = 32*r0 + r1 + r2 in fp8, paired with the [32,1,1,1,1]
    # ones rows of the window tiles
    cnorm = (cbb.astype(np.float64) ** 2).sum(axis=1).astype(np.float32)
    r0 = (cnorm / 32.0).astype(FP8NP)
    rem1 = cnorm - 32.0 * r0.astype(np.float32)
    r1 = rem1.astype(FP8NP)
    rem2 = rem1 - r1.astype(np.float32)
    r2 = rem2.astype(FP8NP)
    rhs8[CHROWS, 0, :] = r0
    rhs8[CHROWS + 1, 0, :] = r1
    rhs8[CHROWS + 2, 0, :] = r2

    ones20 = np.ones((5, NCHUNK * T), dtype=FP8NP)
    ones20[0, :] = FP8NP(32.0)

    # host-side ||f||^2 term via the window-count trick
    tau = np.arange(TP, dtype=np.float64)
    cnt = np.minimum(np.minimum(tau + 1.0, float(WIN)), float(TP) - tau)
    xf = xw.astype(np.float64)
    self_term = float((xf * xf * cnt[None, None, :]).sum())

    in_maps = []
    for i in range(NCORES):
        in_maps.append(
            {
                "xw": np.ascontiguousarray(xw[BC * i : BC * (i + 1)]),
                "cb": rhs8,
                "ones5": ones20,
            }
        )
    return in_maps, self_term


def kernel(x, codebook):
    from concourse.bass_utils import run_bass_kernel_spmd

    nc = get_nc()
    in_maps, self_term = _host_prep(x, codebook)
    res = run_bass_kernel_spmd(nc, in_maps, core_ids=list(range(NCORES)))
    total = np.float64(self_term)
    for r in res.results:
        total += np.float64(r["out"][0, 0])
    return np.array(np.float32(SCALE * total))
